# revision 8
# baseline (speedup 1.0000x reference)
"""Trainium2 Bass kernel for nn_AttentionBlock (multi-head attention block).

Reference computation (fp32):
    q = einsum('bsi,hbik->hbsk', x, Mq)   # Mq: (H,1,I,K) broadcast over b
    k = einsum('bsi,hbik->hbsk', x, Mk)
    v = einsum('bsi,hbiv->hbsv', x, Mv)
    scores  = einsum('hbsk,hbtk->hbst', q, k) / sqrt(K)
    weights = softmax(scores, axis=-1)
    out     = einsum('hbst,hbtv->hbsv', weights, v)   # (H,B,S,V)

Sharding: 8 cores = 4 batches x 2 head-groups (4 heads each). Attention is
independent per (batch, head) so no cross-core communication is needed.

Per-core kernel design (one batch b, 4 heads):
  - xT = x.T via PE transpose (fp32 in, fp16 out)  [I on partitions]
  - QT/KT projections with two heads packed per matmul (lhsT = [Mq_h | Mq_h'],
    128 cols) -> QT/KT packs [128p, S] fp16, head h in partitions 0:64,
    head h' in 64:128.
  - V projection with all 4 heads packed on the moving side (rhs = [Mv_0..Mv_3],
    N=512) -> V natural [t, v] fp16 tiles, with a ones-column appended.
  - scores computed transposed (scoresT[t,s] = k_t . q_s / sqrt(K)) with the two
    heads of a pair issued to disjoint PE row-groups (tile_position) so the
    64-deep contractions run concurrently at full array utilization.
  - exp via ACT directly PSUM -> SBUF fp16 (scale=1/sqrt(K) folded in; softmax
    max-subtraction skipped: logits are O(1) for this problem so exp is safe).
  - AV: out[s, 0:128] and the softmax denominator in one accumulation:
    lhsT = expT chunk [t,128s], rhs = [V | ones] [t, 129]. Column 128 of the
    PSUM result is sum_t exp = denominator, per-partition.
  - evict: out = psum[:, 0:V] * (1/denom) via DVE, DMA to DRAM in natural
    [s, v] layout.
Host side: shard inputs, run SPMD on 8 cores, reassemble (H,B,S,V).
"""

import sys

sys.path.insert(0, "/opt/trn_rl_repo")

import math
from contextlib import ExitStack

import numpy as np

import concourse.bass as bass
import concourse.mybir as mybir
import concourse.tile as tile
from concourse import bacc
from concourse.masks import make_identity

F32 = mybir.dt.float32
F16 = mybir.dt.float16


def build_attention_nc(S=2048, I=1024, K=64, V=128, HPC=4):
    """Build the single-core Bass program (SPMD: same program on all cores)."""
    assert S % 512 == 0 and I % 128 == 0 and V == 128 and K == 64
    assert HPC % 2 == 0
    NSG = S // 512  # s groups of 512 queries
    NST = S // 128  # 128-row tiles (both s and t)
    NCI = I // 128  # contraction chunks for projections
    NPAIR = HPC // 2
    SCALE = 1.0 / math.sqrt(K)

    nc = bacc.Bacc("TRN2", target_bir_lowering=False)
    x = nc.dram_tensor("x", [S, I], F32, kind="ExternalInput")
    mq = nc.dram_tensor("mq", [HPC, I, K], F32, kind="ExternalInput")
    mk = nc.dram_tensor("mk", [HPC, I, K], F32, kind="ExternalInput")
    mv = nc.dram_tensor("mv", [HPC, I, V], F32, kind="ExternalInput")
    out = nc.dram_tensor("out", [HPC, S, V], F32, kind="ExternalOutput")

    with tile.TileContext(nc) as tc, ExitStack() as persist_ctx:
        persist = persist_ctx.enter_context(tc.tile_pool(name="persist", bufs=1))

        # ---------------- persistent SBUF tensors ----------------
        # fp32 identity built on gpsimd, then cast to fp16 on DVE so that every
        # transpose-matmul dependency lives on the DVE semaphore (the S3_LW
        # self-loading matmul encoding only supports a single sync wait).
        ident32 = persist.tile([128, 128], F32, tag="ident32")
        make_identity(nc, ident32)
        ident = persist.tile([128, 128], F16, tag="ident")
        nc.vector.tensor_copy(ident[:], ident32[:])

        xT = persist.tile([128, NCI, S], F16, tag="xT")  # x transposed
        qt = [persist.tile([128, S], F16, tag=f"qt{p}", name=f"qt{p}") for p in range(NPAIR)]
        kt = [persist.tile([128, S], F16, tag=f"kt{p}", name=f"kt{p}") for p in range(NPAIR)]
        # V per head: [t-part, chunk, V+1 (ones) padded]
        vsb = [persist.tile([128, NST, V + 4], F16, tag=f"v{h}", name=f"v{h}") for h in range(HPC)]
        for h in range(HPC):
            nc.vector.memset(vsb[h][:, :, V : V + 1], 1.0)

        mqp = [persist.tile([128, NCI, 128], F16, tag=f"mqp{p}", name=f"mqp{p}") for p in range(NPAIR)]
        mkp = [persist.tile([128, NCI, 128], F16, tag=f"mkp{p}", name=f"mkp{p}") for p in range(NPAIR)]
        mvp = persist.tile([128, NCI, HPC * V], F16, tag="mvp")

        with (
            tc.tile_pool(name="stage", bufs=1) as stage,
            tc.tile_pool(name="xstage", bufs=3) as xstage,
            tc.tile_pool(name="ptr", bufs=2, space="PSUM") as ptr,
            tc.tile_pool(name="pproj", bufs=2, space="PSUM") as pproj,
        ):
            # ------------- phase 0: load + pack + cast weights -------------
            # All weights land in one fp32 staging buffer via disjoint-slice
            # DMAs (no slot reuse -> at most one sync wait per HWDGE DMA).
            WQ, WK, WV = 0, HPC * K, 2 * HPC * K
            wstack = stage.tile([128, NCI, 2 * HPC * K + HPC * V], F32, tag="wstack")
            for h in range(HPC):
                nc.sync.dma_start(
                    wstack[:, :, WQ + h * K : WQ + (h + 1) * K],
                    mq[h].rearrange("(c i) k -> i c k", i=128),
                )
                nc.sync.dma_start(
                    wstack[:, :, WK + h * K : WK + (h + 1) * K],
                    mk[h].rearrange("(c i) k -> i c k", i=128),
                )
                nc.sync.dma_start(
                    wstack[:, :, WV + h * V : WV + (h + 1) * V],
                    mv[h].rearrange("(c i) v -> i c v", i=128),
                )
            for p in range(NPAIR):
                for j in range(2):
                    h = 2 * p + j
                    nc.vector.tensor_copy(
                        mqp[p][:, :, j * K : (j + 1) * K],
                        wstack[:, :, WQ + h * K : WQ + (h + 1) * K],
                    )
                    nc.vector.tensor_copy(
                        mkp[p][:, :, j * K : (j + 1) * K],
                        wstack[:, :, WK + h * K : WK + (h + 1) * K],
                    )
            for h in range(HPC):
                nc.vector.tensor_copy(
                    mvp[:, :, h * V : (h + 1) * V],
                    wstack[:, :, WV + h * V : WV + (h + 1) * V],
                )

            # ------------- phase 1: transpose x via PE -------------
            # x loads go to one persistent fp32 buffer, 8 parallel DMAs into
            # disjoint slices (no slot reuse -> single-wait DMAs). Each 128-row
            # tile is cast to fp16 on DVE, then PE-transposed in fp16.
            xbig = stage.tile([128, NST, I], F32, tag="xbig")
            xr = x.rearrange("(st p) i -> p st i", p=128)
            for u in range(0, NST, 2):
                nc.sync.dma_start(xbig[:, u : u + 2, :], xr[:, u : u + 2, :])
            for st in range(NST):
                xcs = xstage.tile([128, I], F16, tag="xcs")
                nc.vector.tensor_copy(xcs[:], xbig[:, st, :])
                for ci in range(NCI):
                    pt = ptr.tile([128, 128], F16, tag="pt")
                    nc.tensor.transpose(
                        pt[:], xcs[:, ci * 128 : (ci + 1) * 128], ident[:]
                    )
                    nc.vector.tensor_copy(xT[:, ci, st * 128 : (st + 1) * 128], pt[:])

            # ------------- phase 2: projections -------------
            for p in range(NPAIR):
                for sg in range(NSG):
                    psq = pproj.tile([128, 512], F32, tag="psq")
                    psk = pproj.tile([128, 512], F32, tag="psk")
                    for ci in range(NCI):
                        nc.tensor.matmul(
                            psq[:],
                            lhsT=mqp[p][:, ci, :],
                            rhs=xT[:, ci, sg * 512 : (sg + 1) * 512],
                            start=(ci == 0),
                            stop=(ci == NCI - 1),
                        )
                        nc.tensor.matmul(
                            psk[:],
                            lhsT=mkp[p][:, ci, :],
                            rhs=xT[:, ci, sg * 512 : (sg + 1) * 512],
                            start=(ci == 0),
                            stop=(ci == NCI - 1),
                        )
                    nc.vector.tensor_copy(qt[p][:, sg * 512 : (sg + 1) * 512], psq[:])
                    nc.vector.tensor_copy(kt[p][:, sg * 512 : (sg + 1) * 512], psk[:])

            for tt in range(NST):
                psv = pproj.tile([128, HPC * V], F32, tag="psv")
                for ci in range(NCI):
                    nc.tensor.matmul(
                        psv[:],
                        lhsT=xT[:, ci, tt * 128 : (tt + 1) * 128],
                        rhs=mvp[:, ci, :],
                        start=(ci == 0),
                        stop=(ci == NCI - 1),
                    )
                for h in range(HPC):
                    nc.vector.tensor_copy(
                        vsb[h][:, tt, 0:V], psv[:, h * V : (h + 1) * V]
                    )

        # ------------- phase 3: attention -------------
        with (
            tc.tile_pool(name="expp", bufs=2) as expp,
            tc.tile_pool(name="outp", bufs=4) as outp,
            tc.tile_pool(name="recp", bufs=4) as recp,
            tc.tile_pool(name="psc", bufs=2, space="PSUM") as psc,
            tc.tile_pool(name="pav", bufs=4, space="PSUM") as pav,
        ):
            for p in range(NPAIR):
                for sg in range(NSG):
                    # scoresT + exp for both heads of the pair
                    ex = expp.tile([128, NST, 1024], F16, tag="ex")
                    for c in range(NST):
                        ps = psc.tile([128, 1024], F32, tag="ps")
                        for j in range(2):
                            nc.tensor.matmul(
                                ps[:, j * 512 : (j + 1) * 512],
                                lhsT=kt[p][j * 64 : (j + 1) * 64, c * 128 : (c + 1) * 128],
                                rhs=qt[p][j * 64 : (j + 1) * 64, sg * 512 : (sg + 1) * 512],
                                start=True,
                                stop=True,
                                tile_position=(j * 64, 0),
                            )
                        nc.scalar.activation(
                            ex[:, c, :], ps[:], mybir.ActivationFunctionType.Exp,
                            scale=SCALE,
                        )
                    # AV + fused softmax denominator (ones column of vsb)
                    for j in range(2):
                        h = 2 * p + j
                        for stl in range(4):
                            po = pav.tile([128, V + 1], F32, tag="po")
                            soff = j * 512 + stl * 128
                            for c in range(NST):
                                nc.tensor.matmul(
                                    po[:],
                                    lhsT=ex[:, c, soff : soff + 128],
                                    rhs=vsb[h][:, c, 0 : V + 1],
                                    start=(c == 0),
                                    stop=(c == NST - 1),
                                )
                            rec = recp.tile([128, 1], F32, tag="rec")
                            nc.vector.reciprocal(rec[:], po[:, V : V + 1])
                            ob = outp.tile([128, V], F32, tag="ob")
                            nc.vector.tensor_scalar_mul(ob[:], po[:, 0:V], rec[:])
                            row0 = sg * 512 + stl * 128
                            nc.sync.dma_start(out[h, row0 : row0 + 128, :], ob[:])

    nc.compile()
    return nc


_NC_CACHE = {}


def _get_nc():
    if "nc" not in _NC_CACHE:
        _NC_CACHE["nc"] = build_attention_nc()
    return _NC_CACHE["nc"]


def run_sharded(x, Mq, Mk, Mv, **spmd_kwargs):
    """Shard inputs over 8 cores, run, reassemble. Returns (out, BassKernelResults)."""
    from concourse.bass_utils import run_bass_kernel_spmd

    B, S, I = x.shape
    H = Mq.shape[0]
    V = Mv.shape[-1]
    HPC = H // 2  # 4 heads per core, 2 head groups
    x = np.asarray(x, dtype=np.float32)
    Mq = np.asarray(Mq, dtype=np.float32)
    Mk = np.asarray(Mk, dtype=np.float32)
    Mv = np.asarray(Mv, dtype=np.float32)

    in_maps = []
    for c in range(8):
        b, hg = c // 2, c % 2
        hs = slice(hg * HPC, (hg + 1) * HPC)
        in_maps.append(
            {
                "x": np.ascontiguousarray(x[b]),
                "mq": np.ascontiguousarray(Mq[hs, 0]),
                "mk": np.ascontiguousarray(Mk[hs, 0]),
                "mv": np.ascontiguousarray(Mv[hs, 0]),
            }
        )

    nc = _get_nc()
    br = run_bass_kernel_spmd(nc, in_maps, list(range(8)), **spmd_kwargs)

    outf = np.empty((H, B, S, V), dtype=np.float32)
    for c in range(8):
        b, hg = c // 2, c % 2
        outf[hg * HPC : (hg + 1) * HPC, b] = br.results[c]["out"]
    return outf, br


def kernel(x, Mq, Mk, Mv):
    """Full inputs -> full output (H, B, S, V). Shards over 8 NeuronCores."""
    out, _ = run_sharded(x, Mq, Mk, Mv)
    return out


# revision 13
# speedup vs baseline: 17.6333x; 17.6333x over previous
"""Trainium2 Bass kernel for nn_AttentionBlock (multi-head attention block).

Reference computation (fp32):
    q = einsum('bsi,hbik->hbsk', x, Mq)   # Mq: (H,1,I,K) broadcast over b
    k = einsum('bsi,hbik->hbsk', x, Mk)
    v = einsum('bsi,hbiv->hbsv', x, Mv)
    scores  = einsum('hbsk,hbtk->hbst', q, k) / sqrt(K)
    weights = softmax(scores, axis=-1)
    out     = einsum('hbst,hbtv->hbsv', weights, v)   # (H,B,S,V)

Sharding: 8 cores = 4 batches x 2 head-groups (4 heads each). Attention is
independent per (batch, head) so no cross-core communication is needed.

Per-core kernel design (one batch b, 4 heads):
  - xT = x.T via PE transposes in fp16 (x cast on DVE first)  [I on partitions]
  - QT/KT projections with two heads packed per matmul (lhsT = [Mq_h | Mq_h'],
    128 cols) -> QT/KT packs [128p, S] fp16, head h in partitions 0:64,
    head h' in 64:128.
  - V projection with all 4 heads packed on the moving side (rhs = [Mv_0..Mv_3],
    N=512) -> V natural [t, v] fp16 tiles, with a ones-column appended.
  - scores computed transposed (scoresT[t,s] = k_t . q_s / sqrt(K)) with the two
    heads of a pair issued to disjoint PE row-groups (tile_position) so the
    64-deep contractions run concurrently at full array utilization.
  - exp via ACT directly PSUM -> SBUF fp16 (scale=1/sqrt(K) folded in; softmax
    max-subtraction skipped: logits are O(1) for this problem so exp is safe).
  - AV: out[s, 0:128] and the softmax denominator in one accumulation:
    lhsT = expT chunk [t,128s], rhs = [V | ones] [t, 129]. Column 128 of the
    PSUM result is sum_t exp = denominator, per-partition.
  - evict: out = psum[:, 0:V] * (1/denom) via DVE, DMA to DRAM in natural
    [s, v] layout.
Host side: shard inputs, run SPMD on 8 cores, reassemble (H,B,S,V).
"""

import sys

sys.path.insert(0, "/opt/trn_rl_repo")

import math
from contextlib import ExitStack

import numpy as np

import concourse.bass as bass
import concourse.mybir as mybir
import concourse.tile as tile
from concourse import bacc
from concourse.masks import make_identity

F32 = mybir.dt.float32
F16 = mybir.dt.float16


def build_attention_nc(S=2048, I=1024, K=64, V=128, HPC=4, reps=1, tune=None):
    """Build the single-core Bass program (SPMD: same program on all cores).

    reps > 1 re-emits the whole computation (for timing calibration: the
    per-rep delta of one NEFF execution isolates device exec time from
    dispatch overhead).
    """
    assert S % 512 == 0 and I % 128 == 0 and V == 128 and K == 64
    assert HPC % 2 == 0
    NSG = S // 512  # s groups of 512 queries
    NST = S // 128  # 128-row tiles (both s and t)
    NCI = I // 128  # contraction chunks for projections
    NPAIR = HPC // 2
    SCALE = 1.0 / math.sqrt(K)

    nc = bacc.Bacc("TRN2", target_bir_lowering=False)
    x = nc.dram_tensor("x", [S, I], F32, kind="ExternalInput")
    mq = nc.dram_tensor("mq", [HPC, I, K], F32, kind="ExternalInput")
    mk = nc.dram_tensor("mk", [HPC, I, K], F32, kind="ExternalInput")
    mv = nc.dram_tensor("mv", [HPC, I, V], F32, kind="ExternalInput")
    out = nc.dram_tensor("out", [HPC, S, V], F32, kind="ExternalOutput")

    tune = dict(tune or {})
    with tile.TileContext(nc) as tc:
        for rep in range(reps):
            _emit_rep(nc, tc, rep, x, mq, mk, mv, out,
                      S, I, K, V, HPC, NSG, NST, NCI, NPAIR, SCALE, tune)
    nc.compile()
    return nc


def _emit_rep(nc, tc, rep, x, mq, mk, mv, out,
              S, I, K, V, HPC, NSG, NST, NCI, NPAIR, SCALE, tune):
    T = tune.get
    with ExitStack() as persist_ctx:
        persist = persist_ctx.enter_context(
            tc.tile_pool(name=f"persist{rep}", bufs=1)
        )

        # ---------------- persistent SBUF tensors ----------------
        # fp32 identity built on gpsimd, then cast to fp16 on DVE so that every
        # transpose-matmul dependency lives on the DVE semaphore (the S3_LW
        # self-loading matmul encoding only supports a single sync wait).
        ident32 = persist.tile([128, 128], F32, tag="ident32")
        make_identity(nc, ident32)
        ident = persist.tile([128, 128], F16, tag="ident")
        nc.vector.tensor_copy(ident[:], ident32[:])

        xT = persist.tile([128, NCI, S], F16, tag="xT")  # x transposed
        qt = [persist.tile([128, S], F16, tag=f"qt{p}", name=f"qt{rep}_{p}") for p in range(NPAIR)]
        kt = [persist.tile([128, S], F16, tag=f"kt{p}", name=f"kt{rep}_{p}") for p in range(NPAIR)]
        # V per head: [t-part, chunk, V+1 (ones) padded]
        vsb = [persist.tile([128, NST, V + 4], F16, tag=f"v{h}", name=f"v{rep}_{h}") for h in range(HPC)]
        for h in range(HPC):
            nc.vector.memset(vsb[h][:, :, V : V + 1], 1.0)

        mqp = [persist.tile([128, NCI, 128], F16, tag=f"mqp{p}", name=f"mqp{rep}_{p}") for p in range(NPAIR)]
        mkp = [persist.tile([128, NCI, 128], F16, tag=f"mkp{p}", name=f"mkp{rep}_{p}") for p in range(NPAIR)]
        mvp = persist.tile([128, NCI, HPC * V], F16, tag="mvp")

        with (
            tc.tile_pool(name=f"stage{rep}", bufs=1) as stage,
            tc.tile_pool(name=f"xstage{rep}", bufs=T("xstage", 3)) as xstage,
            tc.tile_pool(name=f"ptr{rep}", bufs=T("ptr", 2), space="PSUM") as ptr,
            tc.tile_pool(name=f"pproj{rep}", bufs=T("pproj", 2), space="PSUM") as pproj,
        ):
            # ------------- phase 0: load + pack + cast weights -------------
            # All weights land in one fp32 staging buffer via disjoint-slice
            # DMAs (no slot reuse -> at most one sync wait per HWDGE DMA).
            WQ, WK, WV = 0, HPC * K, 2 * HPC * K
            wstack = stage.tile([128, NCI, 2 * HPC * K + HPC * V], F32, tag="wstack")
            for h in range(HPC):
                nc.sync.dma_start(
                    wstack[:, :, WQ + h * K : WQ + (h + 1) * K],
                    mq[h].rearrange("(c i) k -> i c k", i=128),
                )
                nc.sync.dma_start(
                    wstack[:, :, WK + h * K : WK + (h + 1) * K],
                    mk[h].rearrange("(c i) k -> i c k", i=128),
                )
                nc.sync.dma_start(
                    wstack[:, :, WV + h * V : WV + (h + 1) * V],
                    mv[h].rearrange("(c i) v -> i c v", i=128),
                )
            for p in range(NPAIR):
                for j in range(2):
                    h = 2 * p + j
                    nc.vector.tensor_copy(
                        mqp[p][:, :, j * K : (j + 1) * K],
                        wstack[:, :, WQ + h * K : WQ + (h + 1) * K],
                    )
                    nc.vector.tensor_copy(
                        mkp[p][:, :, j * K : (j + 1) * K],
                        wstack[:, :, WK + h * K : WK + (h + 1) * K],
                    )
            for h in range(HPC):
                nc.vector.tensor_copy(
                    mvp[:, :, h * V : (h + 1) * V],
                    wstack[:, :, WV + h * V : WV + (h + 1) * V],
                )

            # ------------- phase 1: transpose x via PE -------------
            # x loads go to one persistent fp32 buffer, 8 parallel DMAs into
            # disjoint slices (no slot reuse -> single-wait DMAs). Each 128-row
            # tile is cast to fp16 on DVE, then PE-transposed in fp16.
            xbig = stage.tile([128, NST, I], F32, tag="xbig")
            xr = x.rearrange("(st p) i -> p st i", p=128)
            for u in range(0, NST, 2):
                nc.sync.dma_start(xbig[:, u : u + 2, :], xr[:, u : u + 2, :])
            TPK = T("tpack", 1)  # transposes packed per psum tile/eviction
            for st in range(NST):
                xcs = xstage.tile([128, I], F16, tag="xcs")
                nc.vector.tensor_copy(xcs[:], xbig[:, st, :])
                for ci0 in range(0, NCI, TPK):
                    pt = ptr.tile([128, TPK, 128], F16, tag="pt")
                    for j in range(TPK):
                        ci = ci0 + j
                        nc.tensor.transpose(
                            pt[:, j, :], xcs[:, ci * 128 : (ci + 1) * 128], ident[:]
                        )
                    nc.vector.tensor_copy(
                        xT[:, ci0 : ci0 + TPK, st * 128 : (st + 1) * 128], pt[:]
                    )

            # ------------- phase 2: projections -------------
            for p in range(NPAIR):
                for sg in range(NSG):
                    psq = pproj.tile([128, 512], F32, tag="psq")
                    psk = pproj.tile([128, 512], F32, tag="psk")
                    for ci in range(NCI):
                        nc.tensor.matmul(
                            psq[:],
                            lhsT=mqp[p][:, ci, :],
                            rhs=xT[:, ci, sg * 512 : (sg + 1) * 512],
                            start=(ci == 0),
                            stop=(ci == NCI - 1),
                        )
                        nc.tensor.matmul(
                            psk[:],
                            lhsT=mkp[p][:, ci, :],
                            rhs=xT[:, ci, sg * 512 : (sg + 1) * 512],
                            start=(ci == 0),
                            stop=(ci == NCI - 1),
                        )
                    nc.vector.tensor_copy(qt[p][:, sg * 512 : (sg + 1) * 512], psq[:])
                    nc.vector.tensor_copy(kt[p][:, sg * 512 : (sg + 1) * 512], psk[:])

            for tt in range(NST):
                psv = pproj.tile([128, HPC * V], F32, tag="psv")
                for ci in range(NCI):
                    nc.tensor.matmul(
                        psv[:],
                        lhsT=xT[:, ci, tt * 128 : (tt + 1) * 128],
                        rhs=mvp[:, ci, :],
                        start=(ci == 0),
                        stop=(ci == NCI - 1),
                    )
                for h in range(HPC):
                    nc.vector.tensor_copy(
                        vsb[h][:, tt, 0:V], psv[:, h * V : (h + 1) * V]
                    )

        # ------------- phase 3: attention -------------
        with (
            tc.tile_pool(name=f"expp{rep}", bufs=T("expp", 2)) as expp,
            tc.tile_pool(name=f"outp{rep}", bufs=T("outp", 4)) as outp,
            tc.tile_pool(name=f"recp{rep}", bufs=T("recp", 4)) as recp,
            tc.tile_pool(name=f"psc{rep}", bufs=T("psc", 2), space="PSUM") as psc,
            tc.tile_pool(name=f"pav{rep}", bufs=T("pav", 4), space="PSUM") as pav,
        ):
            for p in range(NPAIR):
                for sg in range(NSG):
                    # scoresT + exp for both heads of the pair
                    ECH = T("ech", 1)  # chunks per ACT exp op
                    ex = expp.tile([128, NST, 1024], F16, tag="ex")
                    for c0 in range(0, NST, ECH):
                        ps = psc.tile([128, ECH, 1024], F32, tag="ps")
                        for cj in range(ECH):
                            c = c0 + cj
                            for j in range(2):
                                nc.tensor.matmul(
                                    ps[:, cj, j * 512 : (j + 1) * 512],
                                    lhsT=kt[p][j * 64 : (j + 1) * 64, c * 128 : (c + 1) * 128],
                                    rhs=qt[p][j * 64 : (j + 1) * 64, sg * 512 : (sg + 1) * 512],
                                    start=True,
                                    stop=True,
                                    tile_position=(j * 64, 0),
                                )
                        nc.scalar.activation(
                            ex[:, c0 : c0 + ECH, :], ps[:],
                            mybir.ActivationFunctionType.Exp,
                            scale=SCALE,
                        )
                    # AV + fused softmax denominator (ones column of vsb)
                    for j in range(2):
                        h = 2 * p + j
                        for stl in range(4):
                            po = pav.tile([128, V + 1], F32, tag="po")
                            soff = j * 512 + stl * 128
                            for c in range(NST):
                                nc.tensor.matmul(
                                    po[:],
                                    lhsT=ex[:, c, soff : soff + 128],
                                    rhs=vsb[h][:, c, 0 : V + 1],
                                    start=(c == 0),
                                    stop=(c == NST - 1),
                                )
                            rec = recp.tile([128, 1], F32, tag="rec")
                            nc.vector.reciprocal(rec[:], po[:, V : V + 1])
                            ob = outp.tile([128, V], F32, tag="ob")
                            nc.vector.tensor_scalar_mul(ob[:], po[:, 0:V], rec[:])
                            row0 = sg * 512 + stl * 128
                            nc.sync.dma_start(out[h, row0 : row0 + 128, :], ob[:])


_NC_CACHE = {}

# Best-measured tuning (TimelineSim): pack 8 transposes per PSUM tile/eviction.
DEFAULT_TUNE = {"tpack": 8}


def _install_neff_cache():
    """Persistent on-disk NEFF cache keyed on BIR hash. Saves the ~15min
    neuronxcc compile on repeat runs of the same program on this machine."""
    try:
        import hashlib
        import os
        import shutil

        import concourse.bass_utils as bu
        from concourse import bass2jax

        if getattr(bu.compile_bir_kernel, "_is_cached_wrapper", False):
            return
        orig = bu.compile_bir_kernel
        cache_dir = "/root/neffcache"

        def cached(bir_json, tmpdir, neff_name="file.neff"):
            try:
                h = hashlib.sha256(bir_json).hexdigest()[:24]
                cpath = os.path.join(cache_dir, f"{h}.neff")
                if os.path.exists(cpath):
                    dst = os.path.join(tmpdir, neff_name)
                    shutil.copy(cpath, dst)
                    return dst
                p = orig(bir_json, tmpdir, neff_name)
                os.makedirs(cache_dir, exist_ok=True)
                shutil.copy(p, cpath)
                return p
            except OSError:
                return orig(bir_json, tmpdir, neff_name)

        cached._is_cached_wrapper = True
        bu.compile_bir_kernel = cached
        bass2jax.compile_bir_kernel = cached
    except Exception:
        pass


def _get_nc():
    if "nc" not in _NC_CACHE:
        _NC_CACHE["nc"] = build_attention_nc(tune=DEFAULT_TUNE)
    return _NC_CACHE["nc"]


def run_sharded(x, Mq, Mk, Mv, **spmd_kwargs):
    """Shard inputs over 8 cores, run, reassemble. Returns (out, BassKernelResults)."""
    _install_neff_cache()
    from concourse.bass_utils import run_bass_kernel_spmd

    B, S, I = x.shape
    H = Mq.shape[0]
    V = Mv.shape[-1]
    HPC = H // 2  # 4 heads per core, 2 head groups
    x = np.asarray(x, dtype=np.float32)
    Mq = np.asarray(Mq, dtype=np.float32)
    Mk = np.asarray(Mk, dtype=np.float32)
    Mv = np.asarray(Mv, dtype=np.float32)

    in_maps = []
    for c in range(8):
        b, hg = c // 2, c % 2
        hs = slice(hg * HPC, (hg + 1) * HPC)
        in_maps.append(
            {
                "x": np.ascontiguousarray(x[b]),
                "mq": np.ascontiguousarray(Mq[hs, 0]),
                "mk": np.ascontiguousarray(Mk[hs, 0]),
                "mv": np.ascontiguousarray(Mv[hs, 0]),
            }
        )

    nc = _get_nc()
    br = run_bass_kernel_spmd(nc, in_maps, list(range(8)), **spmd_kwargs)

    outf = np.empty((H, B, S, V), dtype=np.float32)
    for c in range(8):
        b, hg = c // 2, c % 2
        outf[hg * HPC : (hg + 1) * HPC, b] = br.results[c]["out"]
    return outf, br


def kernel(x, Mq, Mk, Mv):
    """Full inputs -> full output (H, B, S, V). Shards over 8 NeuronCores."""
    out, _ = run_sharded(x, Mq, Mk, Mv)
    return out


# revision 21
# speedup vs baseline: 23.7919x; 1.3493x over previous
"""Trainium2 Bass kernel for nn_AttentionBlock (multi-head attention block).

Reference computation (fp32):
    q = einsum('bsi,hbik->hbsk', x, Mq)   # Mq: (H,1,I,K) broadcast over b
    k = einsum('bsi,hbik->hbsk', x, Mk)
    v = einsum('bsi,hbiv->hbsv', x, Mv)
    scores  = einsum('hbsk,hbtk->hbst', q, k) / sqrt(K)
    weights = softmax(scores, axis=-1)
    out     = einsum('hbst,hbtv->hbsv', weights, v)   # (H,B,S,V)

Sharding: 8 cores = 4 batches x 2 head-groups (4 heads each). Attention is
independent per (batch, head) so no cross-core communication is needed.

Per-core kernel design (one batch b, 4 heads):
  - xT = x.T via PE transposes in fp16 (x cast on DVE first)  [I on partitions]
  - QT/KT projections with two heads packed per matmul (lhsT = [Mq_h | Mq_h'],
    128 cols) -> QT/KT packs [128p, S] fp16, head h in partitions 0:64,
    head h' in 64:128.
  - V projection with all 4 heads packed on the moving side (rhs = [Mv_0..Mv_3],
    N=512) -> V natural [t, v] fp16 tiles, with a ones-column appended.
  - scores computed transposed (scoresT[t,s] = k_t . q_s / sqrt(K)) with the two
    heads of a pair issued to disjoint PE row-groups (tile_position) so the
    64-deep contractions run concurrently at full array utilization.
  - exp via ACT directly PSUM -> SBUF fp16 (scale=1/sqrt(K) folded in; softmax
    max-subtraction skipped: logits are O(1) for this problem so exp is safe).
  - AV: out[s, 0:128] and the softmax denominator in one accumulation:
    lhsT = expT chunk [t,128s], rhs = [V | ones] [t, 129]. Column 128 of the
    PSUM result is sum_t exp = denominator, per-partition.
  - evict: out = psum[:, 0:V] * (1/denom) via DVE, DMA to DRAM in natural
    [s, v] layout.
Host side: shard inputs, run SPMD on 8 cores, reassemble (H,B,S,V).
"""

import sys

sys.path.insert(0, "/opt/trn_rl_repo")

import math
from contextlib import ExitStack

import numpy as np

import concourse.bass as bass
import concourse.mybir as mybir
import concourse.tile as tile
from concourse import bacc
from concourse.masks import make_identity

F32 = mybir.dt.float32
F16 = mybir.dt.float16


def build_attention_nc(S=2048, I=1024, K=64, V=128, HPC=4, reps=1, tune=None):
    """Build the single-core Bass program (SPMD: same program on all cores).

    reps > 1 re-emits the whole computation (for timing calibration: the
    per-rep delta of one NEFF execution isolates device exec time from
    dispatch overhead).
    """
    assert S % 512 == 0 and I % 128 == 0 and V == 128 and K == 64
    assert HPC % 2 == 0
    NSG = S // 512  # s groups of 512 queries
    NST = S // 128  # 128-row tiles (both s and t)
    NCI = I // 128  # contraction chunks for projections
    NPAIR = HPC // 2
    SCALE = 1.0 / math.sqrt(K)

    nc = bacc.Bacc("TRN2", target_bir_lowering=False)
    x = nc.dram_tensor("x", [S, I], F32, kind="ExternalInput")
    mq = nc.dram_tensor("mq", [HPC, I, K], F32, kind="ExternalInput")
    mk = nc.dram_tensor("mk", [HPC, I, K], F32, kind="ExternalInput")
    mv = nc.dram_tensor("mv", [HPC, I, V], F32, kind="ExternalInput")
    out = nc.dram_tensor("out", [HPC, S, V], F32, kind="ExternalOutput")

    tune = dict(tune or {})
    with tile.TileContext(nc) as tc:
        for rep in range(reps):
            _emit_rep(nc, tc, rep, x, mq, mk, mv, out,
                      S, I, K, V, HPC, NSG, NST, NCI, NPAIR, SCALE, tune)
    nc.compile()
    return nc


def _emit_rep(nc, tc, rep, x, mq, mk, mv, out,
              S, I, K, V, HPC, NSG, NST, NCI, NPAIR, SCALE, tune):
    T = tune.get
    if T("act_evict", 0):
        def ev_copy(dst, src):
            nc.scalar.copy(dst, src)
    else:
        def ev_copy(dst, src):
            nc.vector.tensor_copy(dst, src)
    with ExitStack() as persist_ctx:
        persist = persist_ctx.enter_context(
            tc.tile_pool(name=f"persist{rep}", bufs=1)
        )

        # ---------------- persistent SBUF tensors ----------------
        # fp32 identity built on gpsimd, then cast to fp16 on DVE so that every
        # transpose-matmul dependency lives on the DVE semaphore (the S3_LW
        # self-loading matmul encoding only supports a single sync wait).
        ident32 = persist.tile([128, 128], F32, tag="ident32")
        make_identity(nc, ident32)
        ident = persist.tile([128, 128], F16, tag="ident")
        ev_copy(ident[:], ident32[:])

        xT = persist.tile([128, NCI, S], F16, tag="xT")  # x transposed
        qt = [persist.tile([128, S], F16, tag=f"qt{p}", name=f"qt{rep}_{p}") for p in range(NPAIR)]
        kt = [persist.tile([128, S], F16, tag=f"kt{p}", name=f"kt{rep}_{p}") for p in range(NPAIR)]
        # V per head: [t-part, chunk, V+1 (ones) padded]
        vsb = [persist.tile([128, NST, V + 4], F16, tag=f"v{h}", name=f"v{rep}_{h}") for h in range(HPC)]
        for h in range(HPC):
            nc.vector.memset(vsb[h][:, :, V : V + 1], 1.0)

        mqp = [persist.tile([128, NCI, 128], F16, tag=f"mqp{p}", name=f"mqp{rep}_{p}") for p in range(NPAIR)]
        mkp = [persist.tile([128, NCI, 128], F16, tag=f"mkp{p}", name=f"mkp{rep}_{p}") for p in range(NPAIR)]
        mvp = persist.tile([128, NCI, HPC * V], F16, tag="mvp")

        FUSE = T("fuse", 0)
        stage_ctx = ExitStack()   # SBUF staging; always closed after phase 2
        psum_ctx = ExitStack()    # projection-phase PSUM pools
        stage = stage_ctx.enter_context(tc.tile_pool(name=f"stage{rep}", bufs=1))
        xstage = stage_ctx.enter_context(
            tc.tile_pool(name=f"xstage{rep}", bufs=T("xstage", 3))
        )
        pproj = psum_ctx.enter_context(
            tc.tile_pool(name=f"pproj{rep}", bufs=T("pproj", 2), space="PSUM")
        )
        ptr_ctx = ExitStack()
        ptr = ptr_ctx.enter_context(
            tc.tile_pool(name=f"ptr{rep}", bufs=T("ptr", 1 if FUSE else 2), space="PSUM")
        )
        # under FUSE, projection psum tiles share one tag (2 banks total) and
        # the attention-phase pools are opened alongside so PSUM fits in 8
        # banks concurrently -> the scheduler can overlap pair-0 scores/exp
        # with the tail of the projection phase.
        ptag = (lambda s: "pp") if FUSE else (lambda s: s)
        if True:
            # ------------- phase 0: load + pack + cast weights -------------
            # All weights land in one fp32 staging buffer via disjoint-slice
            # DMAs (no slot reuse -> at most one sync wait per HWDGE DMA).
            WQ, WK, WV = 0, HPC * K, 2 * HPC * K
            wstack = stage.tile([128, NCI, 2 * HPC * K + HPC * V], F32, tag="wstack")
            xbig = stage.tile([128, NST, I], F32, tag="xbig")
            xr = x.rearrange("(st p) i -> p st i", p=128)
            XS = T("xsplit", 2)  # st-tiles per x DMA
            if T("dma_first", 0):
                nc.sync.dma_start(xbig[:, 0:XS, :], xr[:, 0:XS, :])
            for h in range(HPC):
                nc.sync.dma_start(
                    wstack[:, :, WQ + h * K : WQ + (h + 1) * K],
                    mq[h].rearrange("(c i) k -> i c k", i=128),
                )
                nc.sync.dma_start(
                    wstack[:, :, WK + h * K : WK + (h + 1) * K],
                    mk[h].rearrange("(c i) k -> i c k", i=128),
                )
                nc.sync.dma_start(
                    wstack[:, :, WV + h * V : WV + (h + 1) * V],
                    mv[h].rearrange("(c i) v -> i c v", i=128),
                )
            for p in range(NPAIR):
                for j in range(2):
                    h = 2 * p + j
                    nc.vector.tensor_copy(
                        mqp[p][:, :, j * K : (j + 1) * K],
                        wstack[:, :, WQ + h * K : WQ + (h + 1) * K],
                    )
                    nc.vector.tensor_copy(
                        mkp[p][:, :, j * K : (j + 1) * K],
                        wstack[:, :, WK + h * K : WK + (h + 1) * K],
                    )
            for h in range(HPC):
                nc.vector.tensor_copy(
                    mvp[:, :, h * V : (h + 1) * V],
                    wstack[:, :, WV + h * V : WV + (h + 1) * V],
                )

            # ------------- phase 1: transpose x via PE -------------
            # x loads go to one persistent fp32 buffer, parallel DMAs into
            # disjoint slices (no slot reuse -> single-wait DMAs). Each 128-row
            # tile is cast to fp16, then PE-transposed in fp16.
            for u in range(XS if T("dma_first", 0) else 0, NST, XS):
                nc.sync.dma_start(xbig[:, u : u + XS, :], xr[:, u : u + XS, :])
            TPK = T("tpack", 1)  # transposes packed per psum tile/eviction
            for st in range(NST):
                xcs = xstage.tile([128, I], F16, tag="xcs")
                ev_copy(xcs[:], xbig[:, st, :])
                for ci0 in range(0, NCI, TPK):
                    pt = ptr.tile([128, TPK, 128], F16, tag="pt")
                    for j in range(TPK):
                        ci = ci0 + j
                        nc.tensor.transpose(
                            pt[:, j, :], xcs[:, ci * 128 : (ci + 1) * 128], ident[:]
                        )
                    nc.vector.tensor_copy(
                        xT[:, ci0 : ci0 + TPK, st * 128 : (st + 1) * 128], pt[:]
                    )

            ptr_ctx.close()   # frees the transpose PSUM bank for phase 3
            # ------------- phase 2: projections -------------
            for p in range(NPAIR):
                for sg in range(NSG):
                    psq = pproj.tile([128, 512], F32, tag=ptag("psq"))
                    psk = pproj.tile([128, 512], F32, tag=ptag("psk"))
                    for ci in range(NCI):
                        nc.tensor.matmul(
                            psq[:],
                            lhsT=mqp[p][:, ci, :],
                            rhs=xT[:, ci, sg * 512 : (sg + 1) * 512],
                            start=(ci == 0),
                            stop=(ci == NCI - 1),
                        )
                        nc.tensor.matmul(
                            psk[:],
                            lhsT=mkp[p][:, ci, :],
                            rhs=xT[:, ci, sg * 512 : (sg + 1) * 512],
                            start=(ci == 0),
                            stop=(ci == NCI - 1),
                        )
                    ev_copy(qt[p][:, sg * 512 : (sg + 1) * 512], psq[:])
                    ev_copy(kt[p][:, sg * 512 : (sg + 1) * 512], psk[:])

            def emit_v_proj():
                for tt in range(NST):
                    psv = pproj.tile([128, HPC * V], F32, tag=ptag("psv"), name=f"psv{rep}_{tt}")
                    for ci in range(NCI):
                        nc.tensor.matmul(
                            psv[:],
                            lhsT=xT[:, ci, tt * 128 : (tt + 1) * 128],
                            rhs=mvp[:, ci, :],
                            start=(ci == 0),
                            stop=(ci == NCI - 1),
                        )
                    for h in range(HPC):
                        ev_copy(
                            vsb[h][:, tt, 0:V], psv[:, h * V : (h + 1) * V]
                        )

        # ------------- phase 3: attention -------------
        if not FUSE:
            emit_v_proj()   # pproj pool closes below in this mode
        stage_ctx.close()
        if not FUSE:
            psum_ctx.close()
        att_ctx = ExitStack()
        expp = att_ctx.enter_context(tc.tile_pool(name=f"expp{rep}", bufs=T("expp", 2)))
        outp = att_ctx.enter_context(tc.tile_pool(name=f"outp{rep}", bufs=T("outp", 4)))
        recp = att_ctx.enter_context(tc.tile_pool(name=f"recp{rep}", bufs=T("recp", 4)))
        psc = att_ctx.enter_context(
            tc.tile_pool(name=f"psc{rep}", bufs=T("psc", 2), space="PSUM")
        )
        pav = att_ctx.enter_context(
            tc.tile_pool(name=f"pav{rep}", bufs=T("pav", 2 if FUSE else 4), space="PSUM")
        )
        if True:
            ECH = T("ech", 1)  # chunks per ACT exp op

            def emit_scores_exp(p, sg):
                # scoresT + exp for both heads of the pair; returns the expT tile
                ex = expp.tile([128, NST, 1024], F16, tag="ex", name=f"ex{rep}_{p}_{sg}")
                for c0 in range(0, NST, ECH):
                    ps = psc.tile([128, ECH, 1024], F32, tag="ps", name=f"ps{rep}_{p}_{sg}_{c0}")
                    for cj in range(ECH):
                        c = c0 + cj
                        for j in range(2):
                            nc.tensor.matmul(
                                ps[:, cj, j * 512 : (j + 1) * 512],
                                lhsT=kt[p][j * 64 : (j + 1) * 64, c * 128 : (c + 1) * 128],
                                rhs=qt[p][j * 64 : (j + 1) * 64, sg * 512 : (sg + 1) * 512],
                                start=True,
                                stop=True,
                                tile_position=(j * 64, 0),
                            )
                    nc.scalar.activation(
                        ex[:, c0 : c0 + ECH, :], ps[:],
                        mybir.ActivationFunctionType.Exp,
                        scale=SCALE,
                    )
                return ex

            def emit_av(p, sg, ex):
                # AV + fused softmax denominator (ones column of vsb)
                for j in range(2):
                    h = 2 * p + j
                    for stl in range(4):
                        po = pav.tile([128, V + 1], F32, tag="po", name=f"po{rep}_{p}_{sg}_{j}_{stl}")
                        soff = j * 512 + stl * 128
                        for c in range(NST):
                            nc.tensor.matmul(
                                po[:],
                                lhsT=ex[:, c, soff : soff + 128],
                                rhs=vsb[h][:, c, 0 : V + 1],
                                start=(c == 0),
                                stop=(c == NST - 1),
                            )
                        rec = recp.tile([128, 1], F32, tag="rec", name=f"rec{rep}_{p}_{sg}_{j}_{stl}")
                        nc.vector.reciprocal(rec[:], po[:, V : V + 1])
                        ob = outp.tile([128, V], F32, tag="ob", name=f"ob{rep}_{p}_{sg}_{j}_{stl}")
                        nc.vector.tensor_scalar_mul(ob[:], po[:, 0:V], rec[:])
                        row0 = sg * 512 + stl * 128
                        nc.sync.dma_start(out[h, row0 : row0 + 128, :], ob[:])

            seq = [(p, sg) for p in range(NPAIR) for sg in range(NSG)]
            AHEAD = T("ahead", 0)
            if AHEAD:
                assert FUSE, "ahead requires fuse (pools must coexist)"
                ex_tiles = {}
                for k in range(min(AHEAD, len(seq))):
                    ex_tiles[seq[k]] = emit_scores_exp(*seq[k])
                emit_v_proj()
                for k, (p, sg) in enumerate(seq):
                    emit_av(p, sg, ex_tiles.pop((p, sg)))
                    if k + AHEAD < len(seq):
                        ex_tiles[seq[k + AHEAD]] = emit_scores_exp(*seq[k + AHEAD])
            else:
                if FUSE:
                    emit_v_proj()
                for p, sg in seq:
                    ex = emit_scores_exp(p, sg)
                    emit_av(p, sg, ex)
        att_ctx.close()
        if FUSE:
            psum_ctx.close()


_NC_CACHE = {}

# Best-measured tuning (TimelineSim sweep): pack 8 transposes per PSUM
# tile/eviction; fused PSUM pools + 3-group exp-ahead software pipeline so
# ACT exp (the phase-3 bottleneck) starts during the projection phase;
# per-tile x DMAs for an earlier pipeline start.
DEFAULT_TUNE = {"tpack": 8, "fuse": 1, "ahead": 3, "expp": 3, "xsplit": 1}


def _install_neff_cache():
    """Persistent on-disk NEFF cache keyed on BIR hash. Saves the ~15min
    neuronxcc compile on repeat runs of the same program on this machine."""
    try:
        import hashlib
        import os
        import shutil

        import concourse.bass_utils as bu
        from concourse import bass2jax

        if getattr(bu.compile_bir_kernel, "_is_cached_wrapper", False):
            return
        orig = bu.compile_bir_kernel
        cache_dir = "/root/neffcache"

        def cached(bir_json, tmpdir, neff_name="file.neff"):
            try:
                h = hashlib.sha256(bir_json).hexdigest()[:24]
                cpath = os.path.join(cache_dir, f"{h}.neff")
                if os.path.exists(cpath):
                    dst = os.path.join(tmpdir, neff_name)
                    shutil.copy(cpath, dst)
                    return dst
                p = orig(bir_json, tmpdir, neff_name)
                os.makedirs(cache_dir, exist_ok=True)
                shutil.copy(p, cpath)
                return p
            except OSError:
                return orig(bir_json, tmpdir, neff_name)

        cached._is_cached_wrapper = True
        bu.compile_bir_kernel = cached
        bass2jax.compile_bir_kernel = cached
    except Exception:
        pass


def _get_nc():
    if "nc" not in _NC_CACHE:
        _NC_CACHE["nc"] = build_attention_nc(tune=DEFAULT_TUNE)
    return _NC_CACHE["nc"]


def run_sharded(x, Mq, Mk, Mv, **spmd_kwargs):
    """Shard inputs over 8 cores, run, reassemble. Returns (out, BassKernelResults)."""
    _install_neff_cache()
    from concourse.bass_utils import run_bass_kernel_spmd

    B, S, I = x.shape
    H = Mq.shape[0]
    V = Mv.shape[-1]
    HPC = H // 2  # 4 heads per core, 2 head groups
    x = np.asarray(x, dtype=np.float32)
    Mq = np.asarray(Mq, dtype=np.float32)
    Mk = np.asarray(Mk, dtype=np.float32)
    Mv = np.asarray(Mv, dtype=np.float32)

    in_maps = []
    for c in range(8):
        b, hg = c // 2, c % 2
        hs = slice(hg * HPC, (hg + 1) * HPC)
        in_maps.append(
            {
                "x": np.ascontiguousarray(x[b]),
                "mq": np.ascontiguousarray(Mq[hs, 0]),
                "mk": np.ascontiguousarray(Mk[hs, 0]),
                "mv": np.ascontiguousarray(Mv[hs, 0]),
            }
        )

    nc = _get_nc()
    br = run_bass_kernel_spmd(nc, in_maps, list(range(8)), **spmd_kwargs)

    outf = np.empty((H, B, S, V), dtype=np.float32)
    for c in range(8):
        b, hg = c // 2, c % 2
        outf[hg * HPC : (hg + 1) * HPC, b] = br.results[c]["out"]
    return outf, br


def kernel(x, Mq, Mk, Mv):
    """Full inputs -> full output (H, B, S, V). Shards over 8 NeuronCores."""
    out, _ = run_sharded(x, Mq, Mk, Mv)
    return out
